# revision 11
# baseline (speedup 1.0000x reference)
"""Trainium2 Bass kernel for nn_PairwiseAttentionTerminal.

Reference computation (L=1024, B=8, F=256, H=8, C=32):
    x = layernorm(features)                       # (L, B, F)
    q,k,v = x@Wq+bq, x@Wk+bk, x@Wv+bv             # (L, B, H, C)
    bias  = x@Wb+bb                               # (L, B, H) per-key bias
    gate  = sigmoid(x@Wg+bg)                      # (L, B, H, C)
    S     = einsum('qbhc,kbhc->qbkh', q, k)/sqrt(C) + bias[None]
    attn  = softmax_k(S) @ v                      # (L, B, H, C)
    out   = (attn*gate) @ Wo + bo                 # (L, B, F)

Sharding: batch B=8 -> one batch element per NeuronCore (8 cores), weights
replicated, no collectives.  Host shards/gathers around one SPMD NEFF.

Per-core engine plan (cost-model driven):
  - ACT is the bottleneck: 64 softmax exps of [128k x 1024q] from PSUM
    (per-key bias = per-partition ACT bias operand).  Everything else is
    arranged to hide under that stream.
  - S^T[k,q] per (head, ktile): 32-contraction f32r matmuls (2 x 512 free).
  - AV restructured as out[q, c]: stationary = eT [128k x 128q] slice (bf16),
    moving = ones-augmented V [128k x 33] (bf16) -> 33-cycle matmuls into a
    single-bank accumulator [128, 8qq, 33]; denominator rides along as col 32.
    AV emission is software-pipelined one (h,kk) step behind the S/exp stream
    so the in-order PE queue never blocks on the current exp.
  - Normalize in q-major layout: DVE reciprocal of D*(1+e^-y) fuses the
    sigmoid gate division; no DRAM broadcast roundtrip.  Heads 0-3 normalize
    under the h4-6 exp stream, 4-6 under h7, only h7 in the tail.
  - gate/v/output biases are rank-1 matmuls (ones[1,128] x bias_row) chained
    into the projection accumulation.
  - ag (gated attn, q-major bf16) -> PE-transposed (bf16 identity, 1 cyc/row)
    -> Wo projection per qtile, pipelined drain+DMA tail.
  - PE heater matmuls at t=0 ramp the PE clock (p-state) before the real
    front (LN -> transpose -> q/k projections) hits it.
"""

import numpy as np
from contextlib import ExitStack

L, B, F, H, C = 1024, 8, 256, 8, 32
HC = H * C
C1 = C + 1
EPS = 1e-5
N_CORES = 8
P = 128
NLT = L // P   # 8 L-tiles (== qtiles == ktiles)
NFC = F // P   # 2 F-chunks
NQT = L // P   # 8 q-tiles

_COMPILED = {}


def _build():
    import concourse.bacc as bacc
    import concourse.mybir as mybir
    import concourse.tile as tile

    f32 = mybir.dt.float32
    f32r = mybir.dt.float32r
    bf16 = mybir.dt.bfloat16
    AF = mybir.ActivationFunctionType
    ALU = mybir.AluOpType

    nc = bacc.Bacc("TRN2", target_bir_lowering=False)

    # ---- DRAM I/O (per-core) ----
    feat_e = nc.dram_tensor("feat", [L, F], f32, kind="ExternalInput")
    wq_e = nc.dram_tensor("wq", [P, NFC, HC], f32r, kind="ExternalInput")
    wk_e = nc.dram_tensor("wk", [P, NFC, HC], f32r, kind="ExternalInput")
    wv_e = nc.dram_tensor("wv", [P, NFC, HC], f32r, kind="ExternalInput")
    wg_e = nc.dram_tensor("wg", [P, NFC, HC], f32r, kind="ExternalInput")
    wb_e = nc.dram_tensor("wb", [P, NFC, H], f32r, kind="ExternalInput")
    wo_e = nc.dram_tensor("wo", [P, NFC, F], f32r, kind="ExternalInput")
    bq_e = nc.dram_tensor("bq_t", [P, NFC], f32, kind="ExternalInput")
    bk_e = nc.dram_tensor("bk_t", [P, NFC], f32, kind="ExternalInput")
    bv_e = nc.dram_tensor("bv_r", [1, HC], f32r, kind="ExternalInput")
    bg_e = nc.dram_tensor("bg_r", [1, HC], f32r, kind="ExternalInput")
    bo_e = nc.dram_tensor("bo_r", [1, F], f32r, kind="ExternalInput")
    bb_e = nc.dram_tensor("bb_b", [P, H], f32, kind="ExternalInput")
    id_e = nc.dram_tensor("ident", [P, P], f32r, kind="ExternalInput")
    ones_e = nc.dram_tensor("ones1", [1, P], f32r, kind="ExternalInput")
    out_e = nc.dram_tensor("out", [L, F], f32, kind="ExternalOutput")

    with tile.TileContext(nc) as tc, ExitStack() as ctx:
        const = ctx.enter_context(tc.tile_pool(name="const", bufs=1))
        main = ctx.enter_context(tc.tile_pool(name="main", bufs=1))
        work = ctx.enter_context(tc.tile_pool(name="work", bufs=4))
        epool = ctx.enter_context(tc.tile_pool(name="epool", bufs=3))
        npool = ctx.enter_context(tc.tile_pool(name="npool", bufs=8))
        opool = ctx.enter_context(tc.tile_pool(name="opool", bufs=4))

        # ---- t=0: heater fuel + ACT table prewarm source ----
        ones512 = const.tile([P, 512], f32, name="ones512")
        nc.vector.memset(ones512[:], 1.0)

        # ---- input DMAs, ordered by first use ----
        ft = [const.tile([P, F], f32, name=f"ft{i}") for i in range(NLT)]
        for i in range(4):
            nc.sync.dma_start(ft[i][:], feat_e.ap()[i * P:(i + 1) * P, :])

        def load(eng, name, ext, shape, dt_=f32):
            t = const.tile(shape, dt_, name=name)
            eng.dma_start(t[:], ext.ap())
            return t

        ident = load(nc.sync, "id_s", id_e, [P, P], f32r)
        wq = load(nc.sync, "wq_s", wq_e, [P, NFC, HC], f32r)
        wk = load(nc.sync, "wk_s", wk_e, [P, NFC, HC], f32r)
        bq = load(nc.sync, "bq_s", bq_e, [P, NFC])
        bk = load(nc.sync, "bk_s", bk_e, [P, NFC])
        wb = load(nc.sync, "wb_s", wb_e, [P, NFC, H], f32r)
        bbb = load(nc.sync, "bb_s", bb_e, [P, H])
        ones1 = load(nc.sync, "ones1_s", ones_e, [1, P], f32r)
        wv = load(nc.sync, "wv_s", wv_e, [P, NFC, HC], f32r)
        bvr = load(nc.sync, "bv_s", bv_e, [1, HC], f32r)
        wg = load(nc.sync, "wg_s", wg_e, [P, NFC, HC], f32r)
        bgr = load(nc.sync, "bg_s", bg_e, [1, HC], f32r)
        wo = load(nc.sync, "wo_s", wo_e, [P, NFC, F], f32r)
        bor = load(nc.sync, "bo_s", bo_e, [1, F], f32r)
        for i in range(4, NLT):
            nc.gpsimd.dma_start(ft[i][:], feat_e.ap()[i * P:(i + 1) * P, :])

        # ACT table prewarm: one Ln on the memset-ones tile loads the
        # combined ln/exp table before the front needs it.
        scr0 = const.tile([P, 2], f32, name="scr0")
        nc.scalar.activation(scr0[:, 0:1], ones512[:, 0:1], AF.Ln)

        epst = const.tile([P, 1], f32, name="epst")
        nc.vector.memset(epst[:], EPS)

        # bf16 identity for the ag transposes (1 cyc/row vs 1.5 for f32r)
        identb = const.tile([P, P], bf16, name="identb")
        nc.vector.tensor_copy(identb[:], ident[:])

        # ---- persistent tiles ----
        xT = [main.tile([P, L], f32r, name=f"xT{j}") for j in range(NFC)]
        qT = [main.tile([P, L], f32r, name=f"qT{j}") for j in range(NFC)]
        kT = [main.tile([P, L], f32r, name=f"kT{j}") for j in range(NFC)]
        vaug = [main.tile([P, H, C1], bf16, name=f"vaug{i}") for i in range(NLT)]
        bT = [main.tile([P, H], f32, name=f"bT{i}") for i in range(NLT)]
        ge = [main.tile([P, HC], bf16, name=f"ge{q}") for q in range(NQT)]
        att = main.tile([P, NQT, H, C1], f32, name="att")
        ag = [main.tile([P, HC], bf16, name=f"ag{q}") for q in range(NQT)]
        agT = [main.tile([P, L], f32r, name=f"agT{j}") for j in range(NFC)]

        # ================= Front phase =================
        psF_cm = tc.tile_pool(name="psF", bufs=2, space="PSUM")
        psF = psF_cm.__enter__()
        psH_cm = tc.tile_pool(name="psH", bufs=1, space="PSUM")
        psH = psH_cm.__enter__()

        def heat(n):
            for _ in range(n):
                hp_ = psH.tile([P, 512], f32, tag="h", name="heat", bufs=1)
                nc.tensor.matmul(hp_[:], ones512[:, 0:P].bitcast(f32r),
                                 ones512[:].bitcast(f32r),
                                 start=True, stop=True)

        heat(7)

        # LN + transpose, per L-tile (bn_stats fuses mean+var in one pass)
        def ln_tile(i):
            st = work.tile([P, 8], f32, tag="st")
            nc.vector.bn_stats(st[:, 0:6], ft[i][:])
            nc.vector.bn_aggr(st[:, 6:8], st[:, 0:6])
            # rstd = exp(-0.5*ln(var+eps)) (stays in the one ln/exp table)
            nc.scalar.activation(st[:, 3:4], st[:, 7:8], AF.Ln, bias=epst[:])
            nc.scalar.activation(st[:, 4:5], st[:, 3:4], AF.Exp, scale=-0.5)
            xn = work.tile([P, F], f32r, tag="xn")
            nc.vector.tensor_scalar(xn[:], ft[i][:], st[:, 6:7], st[:, 4:5],
                                    op0=ALU.subtract, op1=ALU.mult)
            for j in range(NFC):
                tp = psF.tile([P, P], f32r, tag="ftp", name=f"tp{i}_{j}")
                nc.tensor.transpose(tp[:], xn[:, j * P:(j + 1) * P], ident[:])
                # alternate drains ACT/DVE (ACT idle during the front)
                if (2 * i + j) % 2 == 0:
                    nc.scalar.activation(xT[j][:, i * P:(i + 1) * P], tp[:],
                                         AF.Copy)
                else:
                    nc.vector.tensor_copy(xT[j][:, i * P:(i + 1) * P], tp[:])

        for i in range(4):
            ln_tile(i)

        # q/k projections; jh chunk 0 in the front, chunk 1 deferred
        def qk_m(j, m, pool, tag):
            ms = slice(512 * m, 512 * (m + 1))
            for (w, bvec, dst) in ((wq, bq, qT), (wk, bk, kT)):
                ps = pool.tile([P, 512], f32, tag=tag, name=f"p{j}{m}")
                nc.tensor.matmul(ps[:], w[:, 0, j * P:(j + 1) * P],
                                 xT[0][:, ms], start=True, stop=False)
                nc.tensor.matmul(ps[:], w[:, 1, j * P:(j + 1) * P],
                                 xT[1][:, ms], start=False, stop=True)
                nc.vector.tensor_scalar(dst[j][:, ms], ps[:],
                                        bvec[:, j:j + 1], None, op0=ALU.add)

        def qk_chunk(j, pool, tag):
            qk_m(j, 0, pool, tag)
            qk_m(j, 1, pool, tag)

        # m=0 slice only needs xT columns 0:512 (L-tiles 0-3)
        qk_m(0, 0, psF, "fqk")
        for i in range(4, NLT):
            ln_tile(i)
        qk_m(0, 1, psF, "fqk")

        # per-key bias projection (tiny) - needed by the first exps
        for i in range(NLT):
            ls = slice(i * P, (i + 1) * P)
            ps2 = psF.tile([P, H], f32, tag="fb", name=f"pb{i}", bufs=1)
            nc.tensor.matmul(ps2[:], xT[0][:, ls], wb[:, 0, :],
                             start=True, stop=False)
            nc.tensor.matmul(ps2[:], xT[1][:, ls], wb[:, 1, :],
                             start=False, stop=True)
            nc.vector.tensor_tensor(bT[i][:], ps2[:], bbb[:], op=ALU.add)

        # v projection (ones-augmented, bias via rank-1 matmul)
        def v_tile(i, pool, tag):
            ls = slice(i * P, (i + 1) * P)
            ps = pool.tile([P, HC], f32, tag=tag, name=f"pv{i}")
            nc.tensor.matmul(ps[:], xT[0][:, ls], wv[:, 0, :],
                             start=True, stop=False)
            nc.tensor.matmul(ps[:], xT[1][:, ls], wv[:, 1, :],
                             start=False, stop=False)
            nc.tensor.matmul(ps[:], ones1[0:1, 0:P], bvr[:],
                             start=False, stop=True)
            nc.vector.memset(
                vaug[i][:, :, C:C1].rearrange("p h one -> p (h one)"), 1.0)
            nc.vector.tensor_copy(
                vaug[i][:, :, 0:C],
                ps[:].rearrange("p (h c) -> p h c", h=H))


        def g_tile(q, pool, tag):
            ls = slice(q * P, (q + 1) * P)
            ps = pool.tile([P, HC], f32, tag=tag, name=f"pg{q}")
            nc.tensor.matmul(ps[:], xT[0][:, ls], wg[:, 0, :],
                             start=True, stop=False)
            nc.tensor.matmul(ps[:], xT[1][:, ls], wg[:, 1, :],
                             start=False, stop=False)
            nc.tensor.matmul(ps[:], ones1[0:1, 0:P], bgr[:],
                             start=False, stop=True)
            # ge = exp(-(y+bg)); gate = 1/(1+ge) folded into normalize
            nc.scalar.activation(ge[q][:], ps[:], AF.Exp, scale=-1.0)

        for q in range(NQT):
            g_tile(q, psF, "fsm")
        v_tile(0, psF, "fsm")
        v_tile(1, psF, "fsm")

        psH_cm.__exit__(None, None, None)
        psF_cm.__exit__(None, None, None)

        # ================= Attention (flattened, software-pipelined) ======
        psD_cm = tc.tile_pool(name="psD", bufs=1, space="PSUM")
        psD = psD_cm.__enter__()
        psS_cm = tc.tile_pool(name="psS", bufs=2, space="PSUM")
        psS = psS_cm.__enter__()
        psAV_cm = tc.tile_pool(name="psAV", bufs=1, space="PSUM")
        psAV = psAV_cm.__enter__()

        avp = [None] * H
        ets = {}

        def emit_AV(h, kk):
            if kk == 0:
                avp[h] = psAV.tile([P, NQT, C1], f32, tag="av", name=f"av{h}")
            eT = ets.pop((h, kk))
            for qq in range(NQT):
                # start marks the whole 2KB psum bank pending-zero; exactly
                # one start per head, one stop on the final matmul.
                nc.tensor.matmul(avp[h][:, qq, :],
                                 eT[:, qq * P:(qq + 1) * P],
                                 vaug[kk][:, h, :],
                                 start=(kk == 0 and qq == 0),
                                 stop=(kk == NLT - 1 and qq == NQT - 1))

        def drain_head(h):
            nc.vector.tensor_copy(att[:, :, h, :], avp[h][:])

        seq = [(h, kk) for h in range(H) for kk in range(NLT)]
        for idx, (h, kk) in enumerate(seq):
            jh, ph = h // 4, 32 * (h % 4)
            hp = slice(ph, ph + 32)
            ks = slice(kk * P, (kk + 1) * P)
            sp = psS.tile([P, L], f32, tag="s", name=f"sp{h}_{kk}")
            for m in range(2):
                ms = slice(512 * m, 512 * (m + 1))
                nc.tensor.matmul(sp[:, ms], kT[jh][hp, ks], qT[jh][hp, ms],
                                 start=True, stop=True, tile_position=(ph, 0))
            eT = epool.tile([P, L], bf16, tag="e", name=f"e{h}_{kk}")
            nc.scalar.activation(eT[:], sp[:], AF.Exp, bias=bT[kk][:, h:h + 1])
            ets[(h, kk)] = eT
            # AV one step behind: the in-order PE queue never waits on the
            # exp that was just issued.
            if idx > 0:
                emit_AV(*seq[idx - 1])
            if kk == 0 and h > 0:
                drain_head(h - 1)
            # deferred front work rides the PE/ACT idle slots of h0/h1
            if h == 0 and kk == 1:
                qk_chunk(1, psD, "dqk")
            if h == 0 and 2 <= kk:
                v_tile(kk, psD, "dsm")
            # normalize early heads under the later heads' exp stream
            if h == 4 and kk == 2:
                for q in range(NQT):
                    _norm(nc, mybir, npool, att, ge, ag, q, 0, 4, engine=q % 2)
            if h == 4 and kk == 6:
                # transpose ag[:, 0:128] (heads 0-3) -> agT[0]
                for q in range(NQT):
                    tq = psD.tile([P, P], bf16, tag="dtp", name=f"tq0_{q}")
                    nc.tensor.transpose(tq[:], ag[q][:, 0:P], identb[:])
                    nc.vector.tensor_copy(agT[0][:, q * P:(q + 1) * P], tq[:])
            if h == 7 and kk == 2:
                for q in range(NQT):
                    _norm(nc, mybir, npool, att, ge, ag, q, 4, 3, engine=q % 2)
        emit_AV(*seq[-1])
        drain_head(H - 1)

        # ================= Tail =================
        for q in range(NQT):
            _norm(nc, mybir, npool, att, ge, ag, q, 7, 1, engine=q % 2)
        psAV_cm.__exit__(None, None, None)
        psS_cm.__exit__(None, None, None)
        psO_cm = tc.tile_pool(name="psO", bufs=3, space="PSUM")
        psO = psO_cm.__enter__()
        for q in range(NQT):
            tq = psO.tile([P, P], bf16, tag="ot", name=f"tq1_{q}", bufs=2)
            nc.tensor.transpose(tq[:], ag[q][:, P:2 * P], identb[:])
            # ACT is free after the last exp
            nc.scalar.activation(agT[1][:, q * P:(q + 1) * P], tq[:],
                                 AF.Copy)
            ls = slice(q * P, (q + 1) * P)
            po = psO.tile([P, F], f32, tag="o", name=f"po{q}")
            nc.tensor.matmul(po[:], agT[0][:, ls], wo[:, 0, :],
                             start=True, stop=False)
            nc.tensor.matmul(po[:], agT[1][:, ls], wo[:, 1, :],
                             start=False, stop=False)
            nc.tensor.matmul(po[:], ones1[0:1, 0:P], bor[:],
                             start=False, stop=True)
            o = opool.tile([P, F], f32, tag="oo", name=f"o{q}")
            if q % 2 == 0:
                nc.scalar.activation(o[:], po[:], AF.Copy)
            else:
                nc.vector.tensor_copy(o[:], po[:])
            (nc.sync if q % 2 == 0 else nc.gpsimd).dma_start(
                out_e.ap()[ls, :], o[:])
        psO_cm.__exit__(None, None, None)
        psD_cm.__exit__(None, None, None)

    # Restrict Exp/Ln/Square to the combined table so one load serves all.
    import concourse.bacc as bacc_mod
    orig_gat = bacc_mod.get_activation_tables
    AFt = mybir.ActivationFunctionType

    def gat_combined(arch):
        t = orig_gat(arch)
        out = {}
        drop = {AFt.Exp, AFt.Ln, AFt.Square}
        for name, funcs in t.items():
            if name == "natural_log_exp_and_others":
                out[name] = funcs
            else:
                out[name] = funcs - drop
        return out

    bacc_mod.get_activation_tables = gat_combined
    try:
        nc.compile()
    finally:
        bacc_mod.get_activation_tables = orig_gat
    return nc


def _norm(nc, mybir, npool, att, ge, ag, q, h0, nh, engine):
    """ag[q][:, h0*32:(h0+nh)*32] = N * 1/(D*(1+ge)) for heads h0..h0+nh-1."""
    ALU = mybir.AluOpType
    f32 = mybir.dt.float32
    bf16 = mybir.dt.bfloat16
    hs = slice(h0 * C, (h0 + nh) * C)
    g1 = npool.tile([P, nh, C], bf16, tag=f"g1_{h0}", name=f"g1_{h0}_{q}")
    dg = npool.tile([P, nh, C], f32, tag=f"dg_{h0}", name=f"dg_{h0}_{q}")
    rc = npool.tile([P, nh, C], f32, tag=f"rc_{h0}", name=f"rc_{h0}_{q}")
    eng = nc.vector if engine == 0 else nc.gpsimd
    gsrc = ge[q][:, hs].rearrange("p (h c) -> p h c", h=nh)
    eng.tensor_scalar(g1[:], gsrc, 1.0, None, op0=ALU.add)
    dsrc = att[:, q, h0:h0 + nh, C:C1].broadcast_to([P, nh, C])
    eng.tensor_tensor(dg[:], dsrc, g1[:], op=ALU.mult)
    nc.vector.reciprocal(rc[:], dg[:])
    nsrc = att[:, q, h0:h0 + nh, 0:C]
    dst = ag[q][:, hs].rearrange("p (h c) -> p h c", h=nh)
    eng.tensor_tensor(dst, nsrc, rc[:], op=ALU.mult)


def _prep_inputs(features, ln_g, ln_b, Wq, bq, Wk, bk, Wv, bv, Wb, bb,
                 Wg, bg, Wo, bo):
    f32 = np.float32
    sq = f32(1.0 / np.sqrt(C))
    g_ = np.asarray(ln_g, f32)[:, None]
    b_ = np.asarray(ln_b, f32)

    def wsplit(W, n):
        return np.ascontiguousarray(
            np.asarray(W, f32).reshape(NFC, P, n).transpose(1, 0, 2))

    def bsplit(b):
        return np.ascontiguousarray(np.asarray(b, f32).reshape(NFC, P).T)

    Wq_ = np.asarray(Wq, f32) * g_ * sq
    bq_ = (b_ @ (np.asarray(Wq, f32) * sq) + np.asarray(bq, f32) * sq)
    Wk_ = np.asarray(Wk, f32) * g_
    bk_ = b_ @ np.asarray(Wk, f32) + np.asarray(bk, f32)
    Wv_ = np.asarray(Wv, f32) * g_
    bv_ = b_ @ np.asarray(Wv, f32) + np.asarray(bv, f32)
    Wg_ = np.asarray(Wg, f32) * g_
    bg_ = b_ @ np.asarray(Wg, f32) + np.asarray(bg, f32)
    Wb_ = np.asarray(Wb, f32) * g_
    bb_ = b_ @ np.asarray(Wb, f32) + np.asarray(bb, f32)

    common = {
        "wq": wsplit(Wq_, HC),
        "wk": wsplit(Wk_, HC),
        "wv": wsplit(Wv_, HC),
        "wg": wsplit(Wg_, HC),
        "wb": wsplit(Wb_, H),
        "wo": wsplit(Wo, F),
        "bq_t": bsplit(bq_),
        "bk_t": bsplit(bk_),
        "bv_r": np.ascontiguousarray(bv_[None, :]),
        "bg_r": np.ascontiguousarray(bg_[None, :]),
        "bo_r": np.ascontiguousarray(np.asarray(bo, f32)[None, :]),
        "bb_b": np.ascontiguousarray(np.tile(bb_, (P, 1))),
        "ident": np.eye(P, dtype=f32),
        "ones1": np.ones((1, P), f32),
    }
    feats = np.asarray(features, f32)
    in_maps = []
    for c_ in range(N_CORES):
        m = dict(common)
        m["feat"] = np.ascontiguousarray(feats[:, c_, :])
        in_maps.append(m)
    return in_maps


def kernel(**inputs):
    from concourse.bass_utils import run_bass_kernel_spmd

    if "nc" not in _COMPILED:
        _COMPILED["nc"] = _build()
    nc = _COMPILED["nc"]
    in_maps = _prep_inputs(**inputs)
    res = run_bass_kernel_spmd(nc, in_maps, list(range(N_CORES)))
    out = np.stack([res.results[c_]["out"] for c_ in range(N_CORES)], axis=1)
    return np.ascontiguousarray(out.astype(np.float32))


if __name__ == "__main__":
    rng = np.random.default_rng(0)
    ins = {
        "features": rng.standard_normal((L, B, F), dtype=np.float32),
        "ln_g": np.ones(F, np.float32), "ln_b": np.zeros(F, np.float32),
        "Wq": rng.standard_normal((F, HC), dtype=np.float32) * 0.02,
        "bq": np.zeros(HC, np.float32),
        "Wk": rng.standard_normal((F, HC), dtype=np.float32) * 0.02,
        "bk": np.zeros(HC, np.float32),
        "Wv": rng.standard_normal((F, HC), dtype=np.float32) * 0.02,
        "bv": np.zeros(HC, np.float32),
        "Wb": rng.standard_normal((F, H), dtype=np.float32) * 0.02,
        "bb": np.zeros(H, np.float32),
        "Wg": rng.standard_normal((F, HC), dtype=np.float32) * 0.02,
        "bg": np.zeros(HC, np.float32),
        "Wo": rng.standard_normal((HC, F), dtype=np.float32) * 0.02,
        "bo": np.zeros(F, np.float32),
    }
    print(kernel(**ins).shape)
